# revision 2
# baseline (speedup 1.0000x reference)
"""Trainium2 Bass kernel for nn_NonLocalLayer (block-causal multi-head attention
layer with 1x1-conv projections, residual, and BatchNorm3d), distributed over
8 NeuronCores.

Sharding: batch x head-pair. Core c handles batch b = c//4 and heads
(2r, 2r+1) with r = c%4. Every core runs an identical instruction stream
(SPMD); only the input data differs. The output projection mixes heads, so
the per-core partial y is ReduceScattered over each batch group of 4 cores
(scatter along channels), then BatchNorm statistics are AllReduced across the
two cores that hold the same channel slice of the two batches.
"""

import numpy as np

import concourse.bacc as bacc
import concourse.mybir as mybir
from concourse.tile import TileContext
from concourse.bass_utils import run_bass_kernel_spmd

F32 = mybir.dt.float32
BF16 = mybir.dt.bfloat16
AF = mybir.ActivationFunctionType
ALU = mybir.AluOpType

N_CORES = 8
CIN = 256
NTOK = 2048          # 16*16*8 tokens
TEMP = 16.0
BN_EPS = 1e-5
BLK = 128            # tokens per time block (H*W)
ICW = 512            # i-chunk width (tokens)
NIC = NTOK // ICW    # 4 i-chunks
JPB = ICW // BLK     # 4 j-blocks per i-chunk

_cache = {}


def _build():
    nc = bacc.Bacc("TRN2", num_devices=N_CORES)

    xb_d = nc.dram_tensor("xb", [256, NTOK], F32, kind="ExternalInput")
    wkt_d = nc.dram_tensor("wkt", [256, 128], F32, kind="ExternalInput")
    wqt_d = nc.dram_tensor("wqt", [256, 128], F32, kind="ExternalInput")
    wvt_d = nc.dram_tensor("wvt", [256, 128], F32, kind="ExternalInput")
    wot_d = nc.dram_tensor("wot", [128, 256], F32, kind="ExternalInput")
    xsl_d = nc.dram_tensor("xsl", [64, NTOK], F32, kind="ExternalInput")
    bo_d = nc.dram_tensor("bo_sl", [64, 1], F32, kind="ExternalInput")
    gam_d = nc.dram_tensor("gam_sl", [64, 1], F32, kind="ExternalInput")
    bet_d = nc.dram_tensor("bet_sl", [64, 1], F32, kind="ExternalInput")
    out_d = nc.dram_tensor("out_sl", [64, NTOK], F32, kind="ExternalOutput")

    with TileContext(nc) as tc:
        with (
            tc.tile_pool(name="const", bufs=1) as constp,
            tc.tile_pool(name="big", bufs=1) as big,
            tc.tile_pool(name="stage", bufs=2) as stage,
            tc.tile_pool(name="work", bufs=3) as work,
            tc.tile_pool(name="norm", bufs=2) as normp,
            tc.tile_pool(name="dram", bufs=1, space="DRAM") as dram,
        ):
            # ---------- load inputs, cast activations/weights to bf16 ----------
            xbh = []
            for ci in range(2):
                xf = stage.tile([128, NTOK], F32, tag="xstage")
                nc.sync.dma_start(xf[:], xb_d[128 * ci:128 * (ci + 1), :])
                xh = big.tile([128, NTOK], BF16, tag=f"xbh{ci}")
                nc.vector.tensor_copy(xh[:], xf[:])
                xbh.append(xh)

            def load_w2(src, label):
                tiles = []
                for ci in range(2):
                    wf = stage.tile([128, 128], F32, tag="wstage")
                    nc.sync.dma_start(wf[:], src[128 * ci:128 * (ci + 1), :])
                    wh = constp.tile([128, 128], BF16, tag=f"{label}{ci}")
                    nc.vector.tensor_copy(wh[:], wf[:])
                    tiles.append(wh)
                return tiles

            wkth = load_w2(wkt_d, "wk")
            wqth = load_w2(wqt_d, "wq")
            wvth = load_w2(wvt_d, "wv")

            woth = []
            for h in range(2):
                wf = stage.tile([64, 256], F32, tag="wostage")
                nc.sync.dma_start(wf[:], wot_d[64 * h:64 * (h + 1), :])
                wh = constp.tile([64, 256], BF16, tag=f"wo{h}")
                nc.vector.tensor_copy(wh[:], wf[:])
                woth.append(wh)

            xsl = big.tile([64, NTOK], F32, tag="xsl")
            nc.sync.dma_start(xsl[:], xsl_d[:, :])
            bo_sb = constp.tile([64, 1], F32, tag="bo")
            nc.sync.dma_start(bo_sb[:], bo_d[:, :])
            gam_sb = constp.tile([64, 1], F32, tag="gam")
            nc.sync.dma_start(gam_sb[:], gam_d[:, :])
            bet_sb = constp.tile([64, 1], F32, tag="bet")
            nc.sync.dma_start(bet_sb[:], bet_d[:, :])

            # ones: row 64 used as lhsT of the denominator-broadcast matmul
            ones_sb = constp.tile([65, 64], BF16, tag="ones")
            nc.vector.memset(ones_sb[:], 1.0)

            # ---------- projections: K, Q as [2 heads * 64 d, tok]; Vhat ----------
            Ksb = big.tile([128, NTOK], BF16, tag="Ksb")
            Qsb = big.tile([128, NTOK], BF16, tag="Qsb")
            vhat = []
            with tc.tile_pool(name="ppsum", bufs=2, space="PSUM") as ppsum:
                for dst, w in ((Ksb, wkth), (Qsb, wqth)):
                    for nchk in range(4):
                        sl = slice(512 * nchk, 512 * (nchk + 1))
                        ps = ppsum.tile([128, 512], F32, tag="proj")
                        nc.tensor.matmul(ps[:], w[0][:], xbh[0][:, sl],
                                         start=True, stop=False)
                        nc.tensor.matmul(ps[:], w[1][:], xbh[1][:, sl],
                                         start=False, stop=True)
                        nc.vector.tensor_copy(dst[:, sl], ps[:])
                # V^T per j-block, with a ones column appended per head:
                # layout [128 j, 0:64 h0 | 64 ones | 65:129 h1 | 129 ones]
                for jt in range(16):
                    sl = slice(128 * jt, 128 * (jt + 1))
                    pv = ppsum.tile([128, 128], F32, tag="vproj")
                    nc.tensor.matmul(pv[:], xbh[0][:, sl], wvth[0][:],
                                     start=True, stop=False)
                    nc.tensor.matmul(pv[:], xbh[1][:, sl], wvth[1][:],
                                     start=False, stop=True)
                    vt = big.tile([128, 130], BF16, tag=f"vhat{jt}")
                    nc.vector.tensor_copy(vt[:, 0:64], pv[:, 0:64])
                    nc.vector.tensor_copy(vt[:, 65:129], pv[:, 64:128])
                    nc.vector.memset(vt[:, 64:65], 1.0)
                    nc.vector.memset(vt[:, 129:130], 1.0)
                    vhat.append(vt)

            # ---------- attention + output projection ----------
            y_bounce = dram.tile([256, NTOK], F32, tag="ybounce")
            with (
                tc.tile_pool(name="spsum", bufs=2, space="PSUM") as spsum,
                tc.tile_pool(name="vpsum", bufs=2, space="PSUM") as vpsum,
            ):
                for ic in range(NIC):
                    njt = JPB * ic + JPB
                    # vo_ps: [65, 1024] = h0 cols 0:512 | h1 cols 512:1024;
                    # row 64 accumulates the softmax denominators.
                    vo_ps = vpsum.tile([65, 1024], F32, tag="sh")
                    for jt in range(njt):
                        ni = 512 if jt < JPB * ic else 512 - BLK * (jt - JPB * ic)
                        off = 512 - ni
                        ist = ICW * ic + off
                        # S^T tile: h0 in cols [off:512], h1 in [512:512+ni]
                        sps = spsum.tile([128, 1024], F32, tag="s")
                        nc.tensor.matmul(
                            sps[0:128, off:512],
                            Qsb[0:64, 128 * jt:128 * (jt + 1)],
                            Ksb[0:64, ist:ist + ni],
                            start=True, stop=True)
                        nc.tensor.matmul(
                            sps[0:128, 512:512 + ni],
                            Qsb[64:128, 128 * jt:128 * (jt + 1)],
                            Ksb[64:128, ist:ist + ni],
                            start=True, stop=True)
                        pt = work.tile([128, 1024], BF16, tag="pt")
                        nc.scalar.activation(pt[:, off:512 + ni],
                                             sps[:, off:512 + ni],
                                             AF.Exp, scale=1.0 / TEMP)
                        first, last = jt == 0, jt == njt - 1
                        nc.tensor.matmul(
                            vo_ps[0:65, off:512],
                            vhat[jt][:, 0:65],
                            pt[:, off:512],
                            start=first, stop=last)
                        nc.tensor.matmul(
                            vo_ps[0:65, 512 + off:1024],
                            vhat[jt][:, 65:130],
                            pt[:, 512:512 + ni],
                            start=first, stop=last)
                    # normalize: vo[d, i] / vo[64, i], both heads at once
                    vo_f = work.tile([65, 1024], F32, tag="vof")
                    nc.scalar.activation(vo_f[:], vo_ps[:], AF.Copy)
                    rcp = work.tile([65, 1024], F32, tag="rcp")
                    nc.vector.reciprocal_approx_fast(rcp[64:65, :], vo_f[64:65, :])
                    rcpb = work.tile([65, 1024], BF16, tag="rcpb")
                    nc.vector.tensor_copy(rcpb[64:65, :], rcp[64:65, :])
                    lb_ps = vpsum.tile([64, 1024], F32, tag="sh")
                    nc.tensor.matmul(lb_ps[:, 0:512], ones_sb[64:65, :],
                                     rcpb[64:65, 0:512], start=True, stop=True)
                    nc.tensor.matmul(lb_ps[:, 512:1024], ones_sb[64:65, :],
                                     rcpb[64:65, 512:1024], start=True, stop=True)
                    vo01 = work.tile([64, 1024], BF16, tag="vo01")
                    nc.vector.tensor_tensor(vo01[:], vo_f[0:64, :], lb_ps[:],
                                            op=ALU.mult)
                    # output projection: y_partial = Wo_h0 @ Vo_h0 + Wo_h1 @ Vo_h1
                    for o in range(2):
                        yp = vpsum.tile([128, 512], F32, tag="sh")
                        nc.tensor.matmul(yp[:], woth[0][:, 128 * o:128 * (o + 1)],
                                         vo01[:, 0:512], start=True, stop=False)
                        nc.tensor.matmul(yp[:], woth[1][:, 128 * o:128 * (o + 1)],
                                         vo01[:, 512:1024], start=False, stop=True)
                        ysb = work.tile([128, 512], F32, tag="ysb")
                        nc.vector.tensor_copy(ysb[:], yp[:])
                        nc.sync.dma_start(
                            y_bounce[128 * o:128 * (o + 1), ICW * ic:ICW * (ic + 1)],
                            ysb[:])

            # ---------- reduce partial y over the 4 cores of this batch ----------
            yred = dram.tile([64, NTOK], F32, tag="yred")
            nc.gpsimd.collective_compute(
                "ReduceScatter", ALU.add,
                replica_groups=[[0, 1, 2, 3], [4, 5, 6, 7]],
                ins=[y_bounce.opt()], outs=[yred.opt()])

            # ---------- epilogue: bias, relu, skip, BN ----------
            ysb2 = big.tile([64, NTOK], F32, tag="ysb2")
            nc.sync.dma_start(ysb2[:], yred[:])
            osb = big.tile([64, NTOK], F32, tag="osb")
            # relu(y + bo) + x
            nc.vector.tensor_scalar(osb[:], ysb2[:], bo_sb[:, 0:1], 0.0,
                                    op0=ALU.add, op1=ALU.max)
            osb2 = big.tile([64, NTOK], F32, tag="osb2")
            nc.vector.tensor_add(osb2[:], osb[:], xsl[:])
            # per-channel sums of out and out^2 (accumulated along free dim)
            scr = big.tile([64, NTOK], F32, tag="scr")
            s1 = normp.tile([64, 1], F32, tag="s1")
            s2 = normp.tile([64, 1], F32, tag="s2")
            nc.scalar.activation(scr[:], osb2[:], AF.Copy, accum_out=s1[:])
            scr2 = big.tile([64, NTOK], F32, tag="scr2")
            nc.scalar.activation(scr2[:], osb2[:], AF.Square, accum_out=s2[:])
            stats = normp.tile([64, 2], F32, tag="stats")
            nc.vector.tensor_copy(stats[:, 0:1], s1[:])
            nc.vector.tensor_copy(stats[:, 1:2], s2[:])
            stats_in = dram.tile([64, 2], F32, tag="statsin")
            stats_out = dram.tile([64, 2], F32, tag="statsout")
            nc.sync.dma_start(stats_in[:], stats[:])
            nc.gpsimd.collective_compute(
                "AllReduce", ALU.add,
                replica_groups=[[0, 4], [1, 5], [2, 6], [3, 7]],
                ins=[stats_in.opt()], outs=[stats_out.opt()])
            statf = normp.tile([64, 2], F32, tag="statf")
            nc.sync.dma_start(statf[:], stats_out[:])

            def tiny(tag):
                return normp.tile([64, 1], F32, tag=tag, name=tag)

            mean = tiny("mean")
            nc.vector.tensor_scalar_mul(mean[:], statf[:, 0:1], 1.0 / 4096.0)
            ex2 = tiny("ex2")
            nc.vector.tensor_scalar_mul(ex2[:], statf[:, 1:2], 1.0 / 4096.0)
            msq = tiny("msq")
            nc.vector.tensor_mul(msq[:], mean[:], mean[:])
            veps = tiny("veps")
            nc.vector.tensor_sub(veps[:], ex2[:], msq[:])
            nc.vector.tensor_scalar_add(veps[:], veps[:], BN_EPS)
            rinv = tiny("rinv")
            nc.vector.reciprocal_approx_fast(rinv[:], veps[:])
            y0 = tiny("y0")
            nc.scalar.sqrt(y0[:], rinv[:])           # ~rsqrt(var+eps)
            # one Newton step: y = y0 * (1.5 - 0.5 * veps * y0^2)
            t1 = tiny("t1")
            nc.vector.tensor_mul(t1[:], y0[:], y0[:])
            t2 = tiny("t2")
            nc.vector.tensor_mul(t2[:], t1[:], veps[:])
            t3 = tiny("t3")
            nc.vector.tensor_scalar(t3[:], t2[:], -0.5, 1.5,
                                    op0=ALU.mult, op1=ALU.add)
            rsq = tiny("rsq")
            nc.vector.tensor_mul(rsq[:], y0[:], t3[:])
            A = tiny("A")
            nc.vector.tensor_mul(A[:], rsq[:], gam_sb[:])
            mA = tiny("mA")
            nc.vector.tensor_mul(mA[:], mean[:], A[:])
            Bv = tiny("Bv")
            nc.vector.tensor_sub(Bv[:], bet_sb[:], mA[:])
            fin = big.tile([64, NTOK], F32, tag="fin")
            nc.vector.tensor_scalar(fin[:], osb2[:], A[:, 0:1], Bv[:, 0:1],
                                    op0=ALU.mult, op1=ALU.add)
            nc.sync.dma_start(out_d[:], fin[:])

    nc.compile()
    return nc


def _get_nc():
    if "nc" not in _cache:
        _cache["nc"] = _build()
    return _cache["nc"]


def kernel(**inputs):
    x = np.asarray(inputs["x"], np.float32)
    WK = np.asarray(inputs["WK"], np.float32)
    WQ = np.asarray(inputs["WQ"], np.float32)
    WV = np.asarray(inputs["WV"], np.float32)
    Wo = np.asarray(inputs["Wo"], np.float32)
    bo = np.asarray(inputs["bo"], np.float32)
    gamma = np.asarray(inputs["gamma"], np.float32)
    beta = np.asarray(inputs["beta"], np.float32)

    nc = _get_nc()

    in_maps = []
    for c in range(N_CORES):
        b, r = c // 4, c % 4
        xf = np.ascontiguousarray(x[b].reshape(CIN, NTOK))
        fs = slice(128 * r, 128 * (r + 1))
        cs = slice(64 * r, 64 * (r + 1))
        in_maps.append({
            "xb": xf,
            "wkt": np.ascontiguousarray(WK[fs, :].T),
            "wqt": np.ascontiguousarray(WQ[fs, :].T),
            "wvt": np.ascontiguousarray(WV[fs, :].T),
            "wot": np.ascontiguousarray(Wo[:, fs].T),
            "xsl": np.ascontiguousarray(xf[cs, :]),
            "bo_sl": np.ascontiguousarray(bo[cs].reshape(64, 1)),
            "gam_sl": np.ascontiguousarray(gamma[cs].reshape(64, 1)),
            "bet_sl": np.ascontiguousarray(beta[cs].reshape(64, 1)),
        })

    import os
    trace = bool(int(os.environ.get("KERNEL_TRACE", "0")))
    res = run_bass_kernel_spmd(nc, in_maps, core_ids=list(range(N_CORES)),
                               trace=trace)
    _cache["last_result"] = res

    out = np.empty((2, CIN, NTOK), np.float32)
    for c in range(N_CORES):
        b, r = c // 4, c % 4
        out[b, 64 * r:64 * (r + 1), :] = res.results[c]["out_sl"]
    return out.reshape(2, CIN, 16, 16, 8)
